# revision 11
# baseline (speedup 1.0000x reference)
"""Multi-head attention (B=2, S=2048, D=1024, H=16) on 8 Trainium2 NeuronCores.

Sharding: batch x head-group. Core c handles batch c//4 and heads 4*(c%4)..4*(c%4)+3
(column-parallel Wq/Wk/Wv, row-parallel Wo; partial outputs summed on host).

Per-core dataflow (all in "transposed" orientation so the PE contracts naturally):
  Q^T/K^T = W^T @ X^T   (f32r matmuls, full fp32 data at 1 cyc/row)  -> bf16 SBUF
  V^T     = Wv^T @ Xv^T -> PE-transpose -> V natural [s, hd] (+ ones col for sums)
  scores^T[sk,sq] = K_h @ Q_h^T  (bf16, two heads row-packed in the 128-wide PE)
  P^T = exp(scores^T/8) (ACT, psum->sbuf bf16), masked by maskT (DVE bf16 2x)
  attn^T[hd+1,sq] = [V_h|1]^T @ P^T  (ones row accumulates softmax denominators)
  normalize via PE ones-broadcast of 1/sums + DVE multiply -> attnT bf16
  out[sq,do] = attnT^T @ Wo  (bf16) -> DMA out.
"""

import numpy as np
import ml_dtypes

B, S, D, H, HD = 2, 2048, 1024, 16, 64
NCORES = 8
HPC = 4          # heads per core
DH4 = HPC * HD   # 256 projection cols per core
KCP = D // 128   # 8 contraction chunks for projections
SC = S // 512    # 4 sq chunks
KCS = S // 128   # 16 sk chunks

_CACHE = {}


def _build_nc():
    from contextlib import ExitStack

    import concourse.bacc as bacc
    import concourse.tile as tile
    from concourse import mybir
    from concourse.masks import make_identity

    dt = mybir.dt
    AF = mybir.ActivationFunctionType

    nc = bacc.Bacc("TRN2", target_bir_lowering=False, debug=False)

    xT = [
        nc.dram_tensor(n, [D, S], dt.float32r, kind="ExternalInput")
        for n in ("xqT", "xkT", "xvT")
    ]
    maskT_d = nc.dram_tensor("maskT", [S, S], dt.bfloat16, kind="ExternalInput")
    w_d = [
        nc.dram_tensor(n, [D, DH4], dt.float32r, kind="ExternalInput")
        for n in ("wq", "wk", "wv")
    ]
    bqkv_d = nc.dram_tensor("bqkv", [3, DH4], dt.float32, kind="ExternalInput")
    wo_d = nc.dram_tensor("wo", [DH4, D], dt.bfloat16, kind="ExternalInput")
    out_d = nc.dram_tensor("out", [S, D], dt.float32, kind="ExternalOutput")

    with tile.TileContext(nc) as tc, ExitStack() as ctx:
        consts = ctx.enter_context(tc.tile_pool(name="consts", bufs=1))
        wpool = ctx.enter_context(tc.tile_pool(name="wpool", bufs=1))
        bigpool = ctx.enter_context(tc.tile_pool(name="bigpool", bufs=1))
        persist = ctx.enter_context(tc.tile_pool(name="persist", bufs=1))
        xtpool = ctx.enter_context(tc.tile_pool(name="xtpool", bufs=3))
        maskpool = ctx.enter_context(tc.tile_pool(name="maskpool", bufs=2))
        ptpool = ctx.enter_context(tc.tile_pool(name="ptpool", bufs=8))
        smalls = ctx.enter_context(tc.tile_pool(name="smalls", bufs=2))
        outpool = ctx.enter_context(tc.tile_pool(name="outpool", bufs=2))
        psp = ctx.enter_context(tc.tile_pool(name="psp", bufs=2, space="PSUM"))
        pvp = ctx.enter_context(tc.tile_pool(name="pvp", bufs=4, space="PSUM"))

        idn = consts.tile([128, 128], dt.float32)
        make_identity(nc, idn[:, :])
        bias_sb = consts.tile([128, 3, 2], dt.float32)
        nc.sync.dma_start(
            out=bias_sb[:, :, :],
            in_=bqkv_d[:, :].rearrange("t (m p) -> p t m", p=128),
        )
        wo_sb = consts.tile([128, 2, D], dt.bfloat16)
        nc.sync.dma_start(
            out=wo_sb[:, :, :], in_=wo_d[:, :].rearrange("(c p) d -> p c d", p=128)
        )

        w_sb = wpool.tile([128, KCP, 3, DH4], dt.float32r, tag="w")
        for t in range(3):
            nc.sync.dma_start(
                out=w_sb[:, :, t, :],
                in_=w_d[t][:, :].rearrange("(kc p) m -> p kc m", p=128),
            )

        qt = persist.tile([128, 2, S], dt.bfloat16, tag="qt")
        kt = persist.tile([128, 2, S], dt.bfloat16, tag="kt")
        vaug = persist.tile([128, KCS, HPC, HD + 1], dt.bfloat16, tag="vaug")
        attnT = persist.tile([128, 2, S], dt.bfloat16, tag="attnT")
        nc.vector.memset(vaug[:, :, :, HD : HD + 1], 1.0)

        # ---- Phase A: projections Q^T, K^T (bf16) and V^T (f32) ----
        vt = bigpool.tile([128, 2, S], dt.float32, tag="big")
        for t in (1, 2, 0):
            dma_eng = {1: nc.sync, 2: nc.scalar, 0: nc.sync}[t]
            for sc in range(SC):
                ps = psp.tile([128, 1024], dt.float32, tag="ps")
                for kc in range(KCP):
                    xt_t = xtpool.tile([128, 512], dt.float32r, tag="xt")
                    dma_eng.dma_start(
                        out=xt_t[:, :],
                        in_=xT[t][
                            kc * 128 : (kc + 1) * 128, sc * 512 : (sc + 1) * 512
                        ],
                    )
                    for m in range(2):
                        nc.tensor.matmul(
                            ps[:, m * 512 : (m + 1) * 512],
                            lhsT=w_sb[:, kc, t, m * 128 : (m + 1) * 128],
                            rhs=xt_t[:, :],
                            start=(kc == 0),
                            stop=(kc == KCP - 1),
                        )
                for m in range(2):
                    dst = (qt, kt, vt)[t]
                    nc.scalar.activation(
                        out=dst[:, m, sc * 512 : (sc + 1) * 512],
                        in_=ps[:, m * 512 : (m + 1) * 512],
                        func=AF.Identity,
                        bias=bias_sb[:, t, m : m + 1],
                        scale=1.0,
                    )

        # ---- V^T -> V natural (PE transpose 128x128 blocks) ----
        for sb in range(KCS):
            for hb in range(2):
                pst = psp.tile([128, 128], dt.float32, tag="ps")
                nc.tensor.transpose(
                    pst[:, :], vt[:, hb, sb * 128 : (sb + 1) * 128], idn[:, :]
                )
                nc.vector.tensor_copy(
                    out=vaug[:, sb, 2 * hb : 2 * hb + 2, 0:HD],
                    in_=pst[:, :].rearrange("p (h d) -> p h d", h=2),
                )

        # ---- Phase B: attention, per (sq-chunk, head-pair) ----
        for sc in range(SC):
            mtile = maskpool.tile(
                [128, KCS, 512], dt.bfloat16, tag="mk", name=f"mk{sc}"
            )
            nc.gpsimd.dma_start(
                out=mtile[:, :, :],
                in_=maskT_d[:, sc * 512 : (sc + 1) * 512].rearrange(
                    "(kc p) q -> p kc q", p=128
                ),
            )
            for p in range(2):
                pv = [
                    pvp.tile([HD + 1, 512], dt.float32, tag="pv", name=f"pv{i}")
                    for i in range(2)
                ]
                sums_sb = smalls.tile([1, 2, 512], dt.float32, tag="sums")
                recip_sb = smalls.tile([1, 2, 512], dt.float32, tag="recip")
                for kc2 in range(KCS // 2):
                    pt = ptpool.tile([128, 2, 2, 512], dt.bfloat16, tag="pt")
                    for j in range(2):
                        kc = 2 * kc2 + j
                        ps = psp.tile([128, 1024], dt.float32, tag="ps")
                        nc.tensor.matmul(
                            ps[:, 0:512],
                            lhsT=kt[0:64, p, kc * 128 : (kc + 1) * 128],
                            rhs=qt[0:64, p, sc * 512 : (sc + 1) * 512],
                            start=True,
                            stop=True,
                        )
                        nc.tensor.matmul(
                            ps[:, 512:1024],
                            lhsT=kt[64:128, p, kc * 128 : (kc + 1) * 128],
                            rhs=qt[64:128, p, sc * 512 : (sc + 1) * 512],
                            start=True,
                            stop=True,
                            tile_position=(64, 0),
                        )
                        nc.scalar.activation(
                            out=pt[:, j, :, :],
                            in_=ps[:, :].rearrange("s (h q) -> s h q", h=2),
                            func=AF.Exp,
                            scale=0.125,
                        )
                    for i in range(2):
                        nc.vector.tensor_mul(
                            out=pt[:, :, i, :],
                            in0=pt[:, :, i, :],
                            in1=mtile[:, 2 * kc2 : 2 * kc2 + 2, :],
                        )
                    for j in range(2):
                        kc = 2 * kc2 + j
                        for i in range(2):
                            nc.tensor.matmul(
                                pv[i][:, :],
                                lhsT=vaug[:, kc, 2 * p + i, :],
                                rhs=pt[:, j, i, :],
                                start=(kc == 0),
                                stop=(kc == KCS - 1),
                            )
                # softmax denominators -> reciprocal -> PE broadcast -> normalize
                for i in range(2):
                    nc.vector.tensor_copy(
                        out=sums_sb[0:1, i, :], in_=pv[i][HD : HD + 1, :]
                    )
                nc.vector.reciprocal(
                    out=recip_sb[0:1, :, :], in_=sums_sb[0:1, :, :]
                )
                bcs = smalls.tile([128, 1024], dt.float32, tag="bcs")
                nc.gpsimd.partition_broadcast(
                    bcs[:, :], recip_sb[0:1, :, :].rearrange("o h q -> o (h q)")
                )
                for i in range(2):
                    nc.vector.tensor_mul(
                        out=attnT[
                            64 * i : 64 * (i + 1), p, sc * 512 : (sc + 1) * 512
                        ],
                        in0=pv[i][0:HD, :],
                        in1=bcs[0:HD, i * 512 : (i + 1) * 512],
                    )
            # ---- output projection for this sq chunk ----
            for s1 in range(sc * 4, sc * 4 + 4):
                po = psp.tile([128, 1024], dt.float32, tag="ps")
                for c in range(2):
                    for m in range(2):
                        nc.tensor.matmul(
                            po[:, m * 512 : (m + 1) * 512],
                            lhsT=attnT[:, c, s1 * 128 : (s1 + 1) * 128],
                            rhs=wo_sb[:, c, m * 512 : (m + 1) * 512],
                            start=(c == 0),
                            stop=(c == 1),
                        )
                ot = outpool.tile([128, 1024], dt.float32, tag="ot")
                if s1 % 2 == 0:
                    nc.scalar.activation(out=ot[:, :], in_=po[:, :], func=AF.Copy)
                else:
                    nc.vector.tensor_copy(out=ot[:, :], in_=po[:, :])
                nc.gpsimd.dma_start(
                    out=out_d[s1 * 128 : (s1 + 1) * 128, :], in_=ot[:, :]
                )


    nc.compile()
    return nc


def _prep_inputs(query, key_, value, mask, Wq, bq, Wk, bk, Wv, bv, Wo, bo):
    bf16 = ml_dtypes.bfloat16
    f32 = np.float32
    per_batch = []
    for b in range(B):
        per_batch.append(
            {
                "xqT": np.ascontiguousarray(np.asarray(query[b], f32).T),
                "xkT": np.ascontiguousarray(np.asarray(key_[b], f32).T),
                "xvT": np.ascontiguousarray(np.asarray(value[b], f32).T),
                "maskT": np.ascontiguousarray(np.asarray(mask[b, 0]).T).astype(bf16),
            }
        )
    in_maps = []
    for c in range(NCORES):
        b, hq = divmod(c, NCORES // B)
        cs = slice(DH4 * hq, DH4 * (hq + 1))
        m = dict(per_batch[b])
        m["wq"] = np.ascontiguousarray(np.asarray(Wq, f32)[:, cs])
        m["wk"] = np.ascontiguousarray(np.asarray(Wk, f32)[:, cs])
        m["wv"] = np.ascontiguousarray(np.asarray(Wv, f32)[:, cs])
        m["bqkv"] = np.ascontiguousarray(
            np.stack([np.asarray(bq, f32)[cs], np.asarray(bk, f32)[cs],
                      np.asarray(bv, f32)[cs]])
        )
        m["wo"] = np.ascontiguousarray(np.asarray(Wo, f32)[cs, :]).astype(bf16)
        in_maps.append(m)
    return in_maps


def kernel(query, key_, value, mask, Wq, bq, Wk, bk, Wv, bv, Wo, bo):
    from concourse.bass_utils import run_bass_kernel_spmd

    if "nc" not in _CACHE:
        _CACHE["nc"] = _build_nc()
    nc = _CACHE["nc"]

    in_maps = _prep_inputs(
        query, key_, value, mask, Wq, bq, Wk, bk, Wv, bv, Wo, bo
    )
    res = run_bass_kernel_spmd(nc, in_maps, core_ids=list(range(NCORES))).results

    out = np.zeros((B, S, D), np.float32)
    for c in range(NCORES):
        out[c // (NCORES // B)] += res[c]["out"]
    out += np.asarray(bo, np.float32)[None, None, :]
    return out


# revision 12
# speedup vs baseline: 1.1399x; 1.1399x over previous
"""Multi-head attention (B=2, S=2048, D=1024, H=16) on 8 Trainium2 NeuronCores.

Sharding: batch x head-group. Core c handles batch c//4 and heads 4*(c%4)..4*(c%4)+3
(column-parallel Wq/Wk/Wv, row-parallel Wo; partial outputs summed on host).

Per-core dataflow (all in "transposed" orientation so the PE contracts naturally):
  Q^T/K^T = W^T @ X^T   (f32r matmuls, full fp32 data at 1 cyc/row)  -> bf16 SBUF
  V^T     = Wv^T @ Xv^T -> PE-transpose -> V natural [s, hd] (+ ones col for sums)
  scores^T[sk,sq] = K_h @ Q_h^T  (bf16, two heads row-packed in the 128-wide PE)
  P^T = exp(scores^T/8) (ACT, psum->sbuf bf16), masked by maskT (DVE bf16 2x)
  attn^T[hd+1,sq] = [V_h|1]^T @ P^T  (ones row accumulates softmax denominators)
  normalize via PE ones-broadcast of 1/sums + DVE multiply -> attnT bf16
  out[sq,do] = attnT^T @ Wo  (bf16) -> DMA out.
"""

import numpy as np
import ml_dtypes

B, S, D, H, HD = 2, 2048, 1024, 16, 64
NCORES = 8
HPC = 4          # heads per core
DH4 = HPC * HD   # 256 projection cols per core
KCP = D // 128   # 8 contraction chunks for projections
SC = S // 512    # 4 sq chunks
KCS = S // 128   # 16 sk chunks

_CACHE = {}


def _build_nc():
    from contextlib import ExitStack

    import concourse.bacc as bacc
    import concourse.tile as tile
    from concourse import mybir
    from concourse.masks import make_identity

    dt = mybir.dt
    AF = mybir.ActivationFunctionType

    nc = bacc.Bacc("TRN2", target_bir_lowering=False, debug=False)

    xT = [
        nc.dram_tensor(n, [D, S], dt.float32r, kind="ExternalInput")
        for n in ("xqT", "xkT", "xvT")
    ]
    maskT_d = nc.dram_tensor("maskT", [S, S], dt.bfloat16, kind="ExternalInput")
    w_d = [
        nc.dram_tensor(n, [D, DH4], dt.float32r, kind="ExternalInput")
        for n in ("wq", "wk", "wv")
    ]
    bqkv_d = nc.dram_tensor("bqkv", [3, DH4], dt.float32, kind="ExternalInput")
    wo_d = nc.dram_tensor("wo", [DH4, D], dt.bfloat16, kind="ExternalInput")
    out_d = nc.dram_tensor("out", [S, D], dt.float32, kind="ExternalOutput")

    with tile.TileContext(nc) as tc, ExitStack() as ctx:
        consts = ctx.enter_context(tc.tile_pool(name="consts", bufs=1))
        wpool = ctx.enter_context(tc.tile_pool(name="wpool", bufs=1))
        bigpool = ctx.enter_context(tc.tile_pool(name="bigpool", bufs=1))
        persist = ctx.enter_context(tc.tile_pool(name="persist", bufs=1))
        xtpool = ctx.enter_context(tc.tile_pool(name="xtpool", bufs=3))
        maskpool = ctx.enter_context(tc.tile_pool(name="maskpool", bufs=2))
        ptpool = ctx.enter_context(tc.tile_pool(name="ptpool", bufs=8))
        smalls = ctx.enter_context(tc.tile_pool(name="smalls", bufs=2))
        outpool = ctx.enter_context(tc.tile_pool(name="outpool", bufs=2))
        psp = ctx.enter_context(tc.tile_pool(name="psp", bufs=2, space="PSUM"))
        pvp = ctx.enter_context(tc.tile_pool(name="pvp", bufs=2, space="PSUM"))
        pop = ctx.enter_context(tc.tile_pool(name="pop", bufs=1, space="PSUM"))

        idn = consts.tile([128, 128], dt.float32)
        make_identity(nc, idn[:, :])
        bias_sb = consts.tile([128, 3, 2], dt.float32)
        nc.sync.dma_start(
            out=bias_sb[:, :, :],
            in_=bqkv_d[:, :].rearrange("t (m p) -> p t m", p=128),
        )
        wo_sb = consts.tile([128, 2, D], dt.bfloat16)
        nc.sync.dma_start(
            out=wo_sb[:, :, :], in_=wo_d[:, :].rearrange("(c p) d -> p c d", p=128)
        )

        w_sb = wpool.tile([128, KCP, 3, DH4], dt.float32r, tag="w")
        for t in range(3):
            nc.sync.dma_start(
                out=w_sb[:, :, t, :],
                in_=w_d[t][:, :].rearrange("(kc p) m -> p kc m", p=128),
            )

        qt = persist.tile([128, 2, S], dt.bfloat16, tag="qt")
        kt = persist.tile([128, 2, S], dt.bfloat16, tag="kt")
        vaug = persist.tile([128, KCS, HPC, HD + 1], dt.bfloat16, tag="vaug")
        attnT = persist.tile([128, 2, S], dt.bfloat16, tag="attnT")
        nc.vector.memset(vaug[:, :, :, HD : HD + 1], 1.0)

        # ---- Phase A: projections Q^T, K^T (bf16) and V^T (f32) ----
        vt = bigpool.tile([128, 2, S], dt.float32, tag="big")
        for t in (1, 2, 0):
            dma_eng = {1: nc.sync, 2: nc.scalar, 0: nc.sync}[t]
            for sc in range(SC):
                ps = psp.tile([128, 1024], dt.float32, tag="ps")
                for kc in range(KCP):
                    xt_t = xtpool.tile([128, 512], dt.float32r, tag="xt")
                    dma_eng.dma_start(
                        out=xt_t[:, :],
                        in_=xT[t][
                            kc * 128 : (kc + 1) * 128, sc * 512 : (sc + 1) * 512
                        ],
                    )
                    for m in range(2):
                        nc.tensor.matmul(
                            ps[:, m * 512 : (m + 1) * 512],
                            lhsT=w_sb[:, kc, t, m * 128 : (m + 1) * 128],
                            rhs=xt_t[:, :],
                            start=(kc == 0),
                            stop=(kc == KCP - 1),
                        )
                for m in range(2):
                    dst = (qt, kt, vt)[t]
                    nc.scalar.activation(
                        out=dst[:, m, sc * 512 : (sc + 1) * 512],
                        in_=ps[:, m * 512 : (m + 1) * 512],
                        func=AF.Identity,
                        bias=bias_sb[:, t, m : m + 1],
                        scale=1.0,
                    )

        # ---- V^T -> V natural (PE transpose 128x128 blocks) ----
        for sb in range(KCS):
            for hb in range(2):
                pst = psp.tile([128, 128], dt.float32, tag="ps")
                nc.tensor.transpose(
                    pst[:, :], vt[:, hb, sb * 128 : (sb + 1) * 128], idn[:, :]
                )
                nc.vector.tensor_copy(
                    out=vaug[:, sb, 2 * hb : 2 * hb + 2, 0:HD],
                    in_=pst[:, :].rearrange("p (h d) -> p h d", h=2),
                )

        # ---- Phase B: attention, per (sq-chunk, head-pair) ----
        for sc in range(SC):
            mtile = maskpool.tile(
                [128, KCS, 512], dt.bfloat16, tag="mk", name=f"mk{sc}"
            )
            nc.gpsimd.dma_start(
                out=mtile[:, :, :],
                in_=maskT_d[:, sc * 512 : (sc + 1) * 512].rearrange(
                    "(kc p) q -> p kc q", p=128
                ),
            )
            for p in range(2):
                pv = [
                    pvp.tile([HD + 1, 512], dt.float32, tag="pv", name=f"pv{i}")
                    for i in range(2)
                ]
                sums_sb = smalls.tile([1, 2, 512], dt.float32, tag="sums")
                recip_sb = smalls.tile([1, 2, 512], dt.float32, tag="recip")
                for kc2 in range(KCS // 2):
                    pt = ptpool.tile([128, 2, 2, 512], dt.bfloat16, tag="pt")
                    for j in range(2):
                        kc = 2 * kc2 + j
                        ps = psp.tile([128, 1024], dt.float32, tag="ps")
                        nc.tensor.matmul(
                            ps[:, 0:512],
                            lhsT=kt[0:64, p, kc * 128 : (kc + 1) * 128],
                            rhs=qt[0:64, p, sc * 512 : (sc + 1) * 512],
                            start=True,
                            stop=True,
                        )
                        nc.tensor.matmul(
                            ps[:, 512:1024],
                            lhsT=kt[64:128, p, kc * 128 : (kc + 1) * 128],
                            rhs=qt[64:128, p, sc * 512 : (sc + 1) * 512],
                            start=True,
                            stop=True,
                            tile_position=(64, 0),
                        )
                        nc.scalar.activation(
                            out=pt[:, j, :, :],
                            in_=ps[:, :].rearrange("s (h q) -> s h q", h=2),
                            func=AF.Exp,
                            scale=0.125,
                        )
                    for i in range(2):
                        nc.vector.tensor_mul(
                            out=pt[:, :, i, :],
                            in0=pt[:, :, i, :],
                            in1=mtile[:, 2 * kc2 : 2 * kc2 + 2, :],
                        )
                    for j in range(2):
                        kc = 2 * kc2 + j
                        for i in range(2):
                            nc.tensor.matmul(
                                pv[i][:, :],
                                lhsT=vaug[:, kc, 2 * p + i, :],
                                rhs=pt[:, j, i, :],
                                start=(kc == 0),
                                stop=(kc == KCS - 1),
                            )
                # softmax denominators -> reciprocal -> PE broadcast -> normalize
                for i in range(2):
                    nc.vector.tensor_copy(
                        out=sums_sb[0:1, i, :], in_=pv[i][HD : HD + 1, :]
                    )
                nc.scalar.activation(
                    out=recip_sb[0:1, :, :],
                    in_=sums_sb[0:1, :, :],
                    func=AF.Ln,
                )
                nc.scalar.activation(
                    out=recip_sb[0:1, :, :],
                    in_=recip_sb[0:1, :, :],
                    func=AF.Exp,
                    scale=-1.0,
                )
                bcs = smalls.tile([128, 1024], dt.float32, tag="bcs")
                nc.gpsimd.partition_broadcast(
                    bcs[:, :], recip_sb[0:1, :, :].rearrange("o h q -> o (h q)")
                )
                for i in range(2):
                    nc.vector.tensor_mul(
                        out=attnT[
                            64 * i : 64 * (i + 1), p, sc * 512 : (sc + 1) * 512
                        ],
                        in0=pv[i][0:HD, :],
                        in1=bcs[0:HD, i * 512 : (i + 1) * 512],
                    )
            # ---- output projection for this sq chunk ----
            for s1 in range(sc * 4, sc * 4 + 4):
                po = pop.tile([128, 1024], dt.float32, tag="po")
                for c in range(2):
                    for m in range(2):
                        nc.tensor.matmul(
                            po[:, m * 512 : (m + 1) * 512],
                            lhsT=attnT[:, c, s1 * 128 : (s1 + 1) * 128],
                            rhs=wo_sb[:, c, m * 512 : (m + 1) * 512],
                            start=(c == 0),
                            stop=(c == 1),
                        )
                ot = outpool.tile([128, 1024], dt.float32, tag="ot")
                if s1 % 2 == 0:
                    nc.scalar.activation(out=ot[:, :], in_=po[:, :], func=AF.Copy)
                else:
                    nc.vector.tensor_copy(out=ot[:, :], in_=po[:, :])
                nc.gpsimd.dma_start(
                    out=out_d[s1 * 128 : (s1 + 1) * 128, :], in_=ot[:, :]
                )


    nc.compile()
    return nc


def _prep_inputs(query, key_, value, mask, Wq, bq, Wk, bk, Wv, bv, Wo, bo):
    bf16 = ml_dtypes.bfloat16
    f32 = np.float32
    per_batch = []
    for b in range(B):
        per_batch.append(
            {
                "xqT": np.ascontiguousarray(np.asarray(query[b], f32).T),
                "xkT": np.ascontiguousarray(np.asarray(key_[b], f32).T),
                "xvT": np.ascontiguousarray(np.asarray(value[b], f32).T),
                "maskT": np.ascontiguousarray(np.asarray(mask[b, 0]).T).astype(bf16),
            }
        )
    in_maps = []
    for c in range(NCORES):
        b, hq = divmod(c, NCORES // B)
        cs = slice(DH4 * hq, DH4 * (hq + 1))
        m = dict(per_batch[b])
        m["wq"] = np.ascontiguousarray(np.asarray(Wq, f32)[:, cs])
        m["wk"] = np.ascontiguousarray(np.asarray(Wk, f32)[:, cs])
        m["wv"] = np.ascontiguousarray(np.asarray(Wv, f32)[:, cs])
        m["bqkv"] = np.ascontiguousarray(
            np.stack([np.asarray(bq, f32)[cs], np.asarray(bk, f32)[cs],
                      np.asarray(bv, f32)[cs]])
        )
        m["wo"] = np.ascontiguousarray(np.asarray(Wo, f32)[cs, :]).astype(bf16)
        in_maps.append(m)
    return in_maps


def kernel(query, key_, value, mask, Wq, bq, Wk, bk, Wv, bv, Wo, bo):
    from concourse.bass_utils import run_bass_kernel_spmd

    if "nc" not in _CACHE:
        _CACHE["nc"] = _build_nc()
    nc = _CACHE["nc"]

    in_maps = _prep_inputs(
        query, key_, value, mask, Wq, bq, Wk, bk, Wv, bv, Wo, bo
    )
    res = run_bass_kernel_spmd(nc, in_maps, core_ids=list(range(NCORES))).results

    out = np.zeros((B, S, D), np.float32)
    for c in range(NCORES):
        out[c // (NCORES // B)] += res[c]["out"]
    out += np.asarray(bo, np.float32)[None, None, :]
    return out


# revision 13
# speedup vs baseline: 1.2213x; 1.0714x over previous
"""Multi-head attention (B=2, S=2048, D=1024, H=16) on 8 Trainium2 NeuronCores.

Sharding: batch x head-group. Core c handles batch c//4 and heads 4*(c%4)..4*(c%4)+3
(column-parallel Wq/Wk/Wv, row-parallel Wo; partial outputs summed on host).

Per-core dataflow (all in "transposed" orientation so the PE contracts naturally):
  Q^T/K^T = W^T @ X^T   (f32r matmuls, full fp32 data at 1 cyc/row)  -> bf16 SBUF
  V^T     = Wv^T @ Xv^T -> PE-transpose -> V natural [s, hd] (+ ones col for sums)
  scores^T[sk,sq] = K_h @ Q_h^T  (bf16, two heads row-packed in the 128-wide PE)
  P^T = exp(scores^T/8) (ACT, psum->sbuf bf16), masked by maskT (DVE bf16 2x)
  attn^T[hd+1,sq] = [V_h|1]^T @ P^T  (ones row accumulates softmax denominators)
  normalize via PE ones-broadcast of 1/sums + DVE multiply -> attnT bf16
  out[sq,do] = attnT^T @ Wo  (bf16) -> DMA out.
"""

import numpy as np
import ml_dtypes

B, S, D, H, HD = 2, 2048, 1024, 16, 64
NCORES = 8
HPC = 4          # heads per core
DH4 = HPC * HD   # 256 projection cols per core
KCP = D // 128   # 8 contraction chunks for projections
SC = S // 512    # 4 sq chunks
KCS = S // 128   # 16 sk chunks

_CACHE = {}


def _build_nc():
    from contextlib import ExitStack

    import concourse.bacc as bacc
    import concourse.tile as tile
    from concourse import mybir
    
    dt = mybir.dt
    AF = mybir.ActivationFunctionType

    nc = bacc.Bacc("TRN2", target_bir_lowering=False, debug=False)

    xT = [
        nc.dram_tensor(n, [D, S], dt.bfloat16, kind="ExternalInput")
        for n in ("xqT", "xkT", "xvT")
    ]
    maskT_d = nc.dram_tensor("maskT", [S, S], dt.bfloat16, kind="ExternalInput")
    w_d = [
        nc.dram_tensor(n, [D, DH4], dt.bfloat16, kind="ExternalInput")
        for n in ("wq", "wk", "wv")
    ]
    bqkv_d = nc.dram_tensor("bqkv", [2, DH4], dt.float32, kind="ExternalInput")
    wo_d = nc.dram_tensor("wo", [DH4, D], dt.bfloat16, kind="ExternalInput")
    out_d = nc.dram_tensor("out", [S, D], dt.float32, kind="ExternalOutput")

    with tile.TileContext(nc) as tc, ExitStack() as ctx:
        consts = ctx.enter_context(tc.tile_pool(name="consts", bufs=1))
        wpool = ctx.enter_context(tc.tile_pool(name="wpool", bufs=1))
        persist = ctx.enter_context(tc.tile_pool(name="persist", bufs=1))
        xtpool = ctx.enter_context(tc.tile_pool(name="xtpool", bufs=9))
        maskpool = ctx.enter_context(tc.tile_pool(name="maskpool", bufs=2))
        ptpool = ctx.enter_context(tc.tile_pool(name="ptpool", bufs=8))
        smalls = ctx.enter_context(tc.tile_pool(name="smalls", bufs=2))
        outpool = ctx.enter_context(tc.tile_pool(name="outpool", bufs=2))
        psp = ctx.enter_context(tc.tile_pool(name="psp", bufs=2, space="PSUM"))
        pvp = ctx.enter_context(tc.tile_pool(name="pvp", bufs=2, space="PSUM"))
        pop = ctx.enter_context(tc.tile_pool(name="pop", bufs=1, space="PSUM"))

        bias_sb = consts.tile([128, 2, 2], dt.float32)
        nc.sync.dma_start(
            out=bias_sb[:, :, :],
            in_=bqkv_d[:, :].rearrange("t (m p) -> p t m", p=128),
        )
        wo_sb = consts.tile([128, 2, D], dt.bfloat16)
        nc.sync.dma_start(
            out=wo_sb[:, :, :], in_=wo_d[:, :].rearrange("(c p) d -> p c d", p=128)
        )

        w_sb = wpool.tile([128, KCP, 3, DH4], dt.bfloat16, tag="w")
        for t in range(3):
            nc.sync.dma_start(
                out=w_sb[:, :, t, :],
                in_=w_d[t][:, :].rearrange("(kc p) m -> p kc m", p=128),
            )

        qt = persist.tile([128, 2, S], dt.bfloat16, tag="qt")
        kt = persist.tile([128, 2, S], dt.bfloat16, tag="kt")
        vaug = persist.tile([128, KCS, HPC, HD + 1], dt.bfloat16, tag="vaug")
        attnT = persist.tile([128, 2, S], dt.bfloat16, tag="attnT")
        nc.vector.memset(vaug[:, :, :, HD : HD + 1], 1.0)

        # ---- Phase A: K^T, Q^T projections (rhs = X^T tiles) and
        # V in natural layout directly (lhsT = Xv^T tiles, stationary) ----
        for t in (1, 2, 0):
            dma_eng = {1: nc.sync, 2: nc.scalar, 0: nc.sync}[t]
            if t == 2:
                for sc in range(SC):
                    xts = []
                    for kc in range(KCP):
                        xt_t = xtpool.tile(
                            [128, 512], dt.bfloat16, tag="xt", name=f"xv{sc}_{kc}"
                        )
                        dma_eng.dma_start(
                            out=xt_t[:, :],
                            in_=xT[t][
                                kc * 128 : (kc + 1) * 128,
                                sc * 512 : (sc + 1) * 512,
                            ],
                        )
                        xts.append(xt_t)
                    for j in range(4):
                        po_v = pvp.tile([128, DH4], dt.float32, tag="pv")
                        for kc in range(KCP):
                            nc.tensor.matmul(
                                po_v[:, :],
                                lhsT=xts[kc][:, j * 128 : (j + 1) * 128],
                                rhs=w_sb[:, kc, 2, :],
                                start=(kc == 0),
                                stop=(kc == KCP - 1),
                            )
                        nc.vector.tensor_copy(
                            out=vaug[:, sc * 4 + j, :, 0:HD],
                            in_=po_v[:, :].rearrange("p (h d) -> p h d", h=4),
                        )
                continue
            for sc in range(SC):
                ps = psp.tile([128, 1024], dt.float32, tag="ps")
                for kc in range(KCP):
                    xt_t = xtpool.tile([128, 512], dt.bfloat16, tag="xt")
                    dma_eng.dma_start(
                        out=xt_t[:, :],
                        in_=xT[t][
                            kc * 128 : (kc + 1) * 128, sc * 512 : (sc + 1) * 512
                        ],
                    )
                    for m in range(2):
                        nc.tensor.matmul(
                            ps[:, m * 512 : (m + 1) * 512],
                            lhsT=w_sb[:, kc, t, m * 128 : (m + 1) * 128],
                            rhs=xt_t[:, :],
                            start=(kc == 0),
                            stop=(kc == KCP - 1),
                        )
                for m in range(2):
                    dst = (qt, kt)[t]
                    nc.scalar.activation(
                        out=dst[:, m, sc * 512 : (sc + 1) * 512],
                        in_=ps[:, m * 512 : (m + 1) * 512],
                        func=AF.Identity,
                        bias=bias_sb[:, t, m : m + 1],
                        scale=1.0,
                    )

        # ---- Phase B: attention, per (sq-chunk, head-pair) ----
        for sc in range(SC):
            mtile = maskpool.tile(
                [128, KCS, 512], dt.bfloat16, tag="mk", name=f"mk{sc}"
            )
            nc.gpsimd.dma_start(
                out=mtile[:, :, :],
                in_=maskT_d[:, sc * 512 : (sc + 1) * 512].rearrange(
                    "(kc p) q -> p kc q", p=128
                ),
            )
            for p in range(2):
                pv = [
                    pvp.tile([HD + 1, 512], dt.float32, tag="pv", name=f"pv{i}")
                    for i in range(2)
                ]
                sums_sb = smalls.tile([1, 2, 512], dt.float32, tag="sums")
                recip_sb = smalls.tile([1, 2, 512], dt.float32, tag="recip")
                for kc2 in range(KCS // 2):
                    pt = ptpool.tile([128, 2, 2, 512], dt.bfloat16, tag="pt")
                    for j in range(2):
                        kc = 2 * kc2 + j
                        ps = psp.tile([128, 1024], dt.float32, tag="ps")
                        nc.tensor.matmul(
                            ps[:, 0:512],
                            lhsT=kt[0:64, p, kc * 128 : (kc + 1) * 128],
                            rhs=qt[0:64, p, sc * 512 : (sc + 1) * 512],
                            start=True,
                            stop=True,
                        )
                        nc.tensor.matmul(
                            ps[:, 512:1024],
                            lhsT=kt[64:128, p, kc * 128 : (kc + 1) * 128],
                            rhs=qt[64:128, p, sc * 512 : (sc + 1) * 512],
                            start=True,
                            stop=True,
                            tile_position=(64, 0),
                        )
                        nc.scalar.activation(
                            out=pt[:, j, :, :],
                            in_=ps[:, :].rearrange("s (h q) -> s h q", h=2),
                            func=AF.Exp,
                            scale=0.125,
                        )
                    for i in range(2):
                        nc.vector.tensor_mul(
                            out=pt[:, :, i, :],
                            in0=pt[:, :, i, :],
                            in1=mtile[:, 2 * kc2 : 2 * kc2 + 2, :],
                        )
                    for j in range(2):
                        kc = 2 * kc2 + j
                        for i in range(2):
                            nc.tensor.matmul(
                                pv[i][:, :],
                                lhsT=vaug[:, kc, 2 * p + i, :],
                                rhs=pt[:, j, i, :],
                                start=(kc == 0),
                                stop=(kc == KCS - 1),
                            )
                # softmax denominators -> reciprocal -> PE broadcast -> normalize
                for i in range(2):
                    nc.vector.tensor_copy(
                        out=sums_sb[0:1, i, :], in_=pv[i][HD : HD + 1, :]
                    )
                nc.scalar.activation(
                    out=recip_sb[0:1, :, :],
                    in_=sums_sb[0:1, :, :],
                    func=AF.Ln,
                )
                nc.scalar.activation(
                    out=recip_sb[0:1, :, :],
                    in_=recip_sb[0:1, :, :],
                    func=AF.Exp,
                    scale=-1.0,
                )
                bcs = smalls.tile([128, 1024], dt.float32, tag="bcs")
                nc.gpsimd.partition_broadcast(
                    bcs[:, :], recip_sb[0:1, :, :].rearrange("o h q -> o (h q)")
                )
                for i in range(2):
                    nc.vector.tensor_mul(
                        out=attnT[
                            64 * i : 64 * (i + 1), p, sc * 512 : (sc + 1) * 512
                        ],
                        in0=pv[i][0:HD, :],
                        in1=bcs[0:HD, i * 512 : (i + 1) * 512],
                    )
            # ---- output projection for this sq chunk ----
            for s1 in range(sc * 4, sc * 4 + 4):
                po = pop.tile([128, 1024], dt.float32, tag="po")
                for c in range(2):
                    for m in range(2):
                        nc.tensor.matmul(
                            po[:, m * 512 : (m + 1) * 512],
                            lhsT=attnT[:, c, s1 * 128 : (s1 + 1) * 128],
                            rhs=wo_sb[:, c, m * 512 : (m + 1) * 512],
                            start=(c == 0),
                            stop=(c == 1),
                        )
                ot = outpool.tile([128, 1024], dt.float32, tag="ot")
                if s1 % 2 == 0:
                    nc.scalar.activation(out=ot[:, :], in_=po[:, :], func=AF.Copy)
                else:
                    nc.vector.tensor_copy(out=ot[:, :], in_=po[:, :])
                nc.gpsimd.dma_start(
                    out=out_d[s1 * 128 : (s1 + 1) * 128, :], in_=ot[:, :]
                )


    nc.compile()
    return nc


def _prep_inputs(query, key_, value, mask, Wq, bq, Wk, bk, Wv, bv, Wo, bo):
    bf16 = ml_dtypes.bfloat16
    f32 = np.float32
    per_batch = []
    for b in range(B):
        per_batch.append(
            {
                "xqT": np.ascontiguousarray(np.asarray(query[b], f32).T).astype(bf16),
                "xkT": np.ascontiguousarray(np.asarray(key_[b], f32).T).astype(bf16),
                "xvT": np.ascontiguousarray(np.asarray(value[b], f32).T).astype(bf16),
                "maskT": np.ascontiguousarray(np.asarray(mask[b, 0]).T).astype(bf16),
            }
        )
    in_maps = []
    for c in range(NCORES):
        b, hq = divmod(c, NCORES // B)
        cs = slice(DH4 * hq, DH4 * (hq + 1))
        m = dict(per_batch[b])
        m["wq"] = np.ascontiguousarray(np.asarray(Wq, f32)[:, cs]).astype(bf16)
        m["wk"] = np.ascontiguousarray(np.asarray(Wk, f32)[:, cs]).astype(bf16)
        m["wv"] = np.ascontiguousarray(np.asarray(Wv, f32)[:, cs]).astype(bf16)
        m["bqkv"] = np.ascontiguousarray(
            np.stack([np.asarray(bq, f32)[cs], np.asarray(bk, f32)[cs]])
        )
        m["wo"] = np.ascontiguousarray(np.asarray(Wo, f32)[cs, :]).astype(bf16)
        in_maps.append(m)
    return in_maps


def kernel(query, key_, value, mask, Wq, bq, Wk, bk, Wv, bv, Wo, bo):
    from concourse.bass_utils import run_bass_kernel_spmd

    if "nc" not in _CACHE:
        _CACHE["nc"] = _build_nc()
    nc = _CACHE["nc"]

    in_maps = _prep_inputs(
        query, key_, value, mask, Wq, bq, Wk, bk, Wv, bv, Wo, bo
    )
    res = run_bass_kernel_spmd(nc, in_maps, core_ids=list(range(NCORES))).results

    out = np.zeros((B, S, D), np.float32)
    for c in range(NCORES):
        out[c // (NCORES // B)] += res[c]["out"]
    out += (
        np.asarray(bv, np.float32) @ np.asarray(Wo, np.float32)
        + np.asarray(bo, np.float32)
    )[None, None, :]
    return out


# revision 15
# speedup vs baseline: 1.4816x; 1.2131x over previous
"""Multi-head attention (B=2, S=2048, D=1024, H=16) on 8 Trainium2 NeuronCores.

Sharding: batch x head-group. Core c handles batch c//4 and heads 4*(c%4)..4*(c%4)+3
(column-parallel Wq/Wk/Wv, row-parallel Wo; partial outputs summed on host).

Per-core dataflow (all in "transposed" orientation so the PE contracts naturally):
  Q^T/K^T = W^T @ X^T   (f32r matmuls, full fp32 data at 1 cyc/row)  -> bf16 SBUF
  V^T     = Wv^T @ Xv^T -> PE-transpose -> V natural [s, hd] (+ ones col for sums)
  scores^T[sk,sq] = K_h @ Q_h^T  (bf16, two heads row-packed in the 128-wide PE)
  P^T = exp(scores^T/8) (ACT, psum->sbuf bf16), masked by maskT (DVE bf16 2x)
  attn^T[hd+1,sq] = [V_h|1]^T @ P^T  (ones row accumulates softmax denominators)
  normalize via PE ones-broadcast of 1/sums + DVE multiply -> attnT bf16
  out[sq,do] = attnT^T @ Wo  (bf16) -> DMA out.
"""

import numpy as np
import ml_dtypes

B, S, D, H, HD = 2, 2048, 1024, 16, 64
NCORES = 8
HPC = 4          # heads per core
DH4 = HPC * HD   # 256 projection cols per core
KCP = D // 128   # 8 contraction chunks for projections
SC = S // 512    # 4 sq chunks
KCS = S // 128   # 16 sk chunks

_CACHE = {}


def _build_nc():
    from contextlib import ExitStack

    import concourse.bacc as bacc
    import concourse.tile as tile
    from concourse import mybir
    
    dt = mybir.dt
    AF = mybir.ActivationFunctionType

    nc = bacc.Bacc("TRN2", target_bir_lowering=False, debug=False)

    xT = [
        nc.dram_tensor(n, [D, S], dt.bfloat16, kind="ExternalInput")
        for n in ("xqT", "xkT", "xvT")
    ]
    maskT_d = nc.dram_tensor("maskT", [S, S], dt.bfloat16, kind="ExternalInput")
    w_d = [
        nc.dram_tensor(n, [D, DH4], dt.bfloat16, kind="ExternalInput")
        for n in ("wq", "wk", "wv")
    ]
    bqkv_d = nc.dram_tensor("bqkv", [2, DH4], dt.float32, kind="ExternalInput")
    wo_d = nc.dram_tensor("wo", [DH4, D], dt.bfloat16, kind="ExternalInput")
    out_d = nc.dram_tensor("out", [S, D], dt.float32, kind="ExternalOutput")

    with tile.TileContext(nc) as tc, ExitStack() as ctx:
        consts = ctx.enter_context(tc.tile_pool(name="consts", bufs=1))
        wpool = ctx.enter_context(tc.tile_pool(name="wpool", bufs=1))
        persist = ctx.enter_context(tc.tile_pool(name="persist", bufs=1))
        xtpool = ctx.enter_context(tc.tile_pool(name="xtpool", bufs=4))
        xvpool = ctx.enter_context(tc.tile_pool(name="xvpool", bufs=12))
        maskpool = ctx.enter_context(tc.tile_pool(name="maskpool", bufs=2))
        ptpool = ctx.enter_context(tc.tile_pool(name="ptpool", bufs=8))
        smalls = ctx.enter_context(tc.tile_pool(name="smalls", bufs=2))
        outpool = ctx.enter_context(tc.tile_pool(name="outpool", bufs=2))
        psp = ctx.enter_context(tc.tile_pool(name="psp", bufs=2, space="PSUM"))
        pvp = ctx.enter_context(tc.tile_pool(name="pvp", bufs=2, space="PSUM"))
        pop = ctx.enter_context(tc.tile_pool(name="pop", bufs=1, space="PSUM"))

        bias_sb = consts.tile([128, 2, 2], dt.float32)
        nc.sync.dma_start(
            out=bias_sb[:, :, :],
            in_=bqkv_d[:, :].rearrange("t (m p) -> p t m", p=128),
        )
        wo_sb = consts.tile([128, 2, D], dt.bfloat16)
        nc.sync.dma_start(
            out=wo_sb[:, :, :], in_=wo_d[:, :].rearrange("(c p) d -> p c d", p=128)
        )

        w_sb = wpool.tile([128, KCP, 3, DH4], dt.bfloat16, tag="w")
        for t in range(3):
            nc.sync.dma_start(
                out=w_sb[:, :, t, :],
                in_=w_d[t][:, :].rearrange("(kc p) m -> p kc m", p=128),
            )

        qt = persist.tile([128, 2, S], dt.bfloat16, tag="qt")
        kt = persist.tile([128, 2, S], dt.bfloat16, tag="kt")
        vaug = persist.tile([128, KCS, HPC, HD + 1], dt.bfloat16, tag="vaug")
        attnT = persist.tile([128, 2, S], dt.bfloat16, tag="attnT")
        nc.vector.memset(vaug[:, :, :, HD : HD + 1], 1.0)

        # ---- Phase A: K^T, Q^T projections (rhs = X^T tiles) and
        # V in natural layout directly (lhsT = Xv^T tiles, stationary) ----
        for t in (1, 2, 0):
            dma_eng = {1: nc.sync, 2: nc.scalar, 0: nc.sync}[t]
            if t == 2:
                for sc in range(SC):
                    xts = []
                    for kc in range(KCP):
                        xt_t = xvpool.tile(
                            [128, 512], dt.bfloat16, tag="xv", name=f"xv{sc}_{kc}"
                        )
                        dma_eng.dma_start(
                            out=xt_t[:, :],
                            in_=xT[t][
                                kc * 128 : (kc + 1) * 128,
                                sc * 512 : (sc + 1) * 512,
                            ],
                        )
                        xts.append(xt_t)
                    for j in range(4):
                        po_v = pvp.tile([128, DH4], dt.float32, tag="pv")
                        for kc in range(KCP):
                            nc.tensor.matmul(
                                po_v[:, :],
                                lhsT=xts[kc][:, j * 128 : (j + 1) * 128],
                                rhs=w_sb[:, kc, 2, :],
                                start=(kc == 0),
                                stop=(kc == KCP - 1),
                            )
                        nc.vector.tensor_copy(
                            out=vaug[:, sc * 4 + j, :, 0:HD],
                            in_=po_v[:, :].rearrange("p (h d) -> p h d", h=4),
                        )
                continue
            for sc in range(SC):
                ps = psp.tile([128, 1024], dt.float32, tag="ps")
                for kc in range(KCP):
                    xt_t = xtpool.tile([128, 512], dt.bfloat16, tag="xt")
                    dma_eng.dma_start(
                        out=xt_t[:, :],
                        in_=xT[t][
                            kc * 128 : (kc + 1) * 128, sc * 512 : (sc + 1) * 512
                        ],
                    )
                    for m in range(2):
                        nc.tensor.matmul(
                            ps[:, m * 512 : (m + 1) * 512],
                            lhsT=w_sb[:, kc, t, m * 128 : (m + 1) * 128],
                            rhs=xt_t[:, :],
                            start=(kc == 0),
                            stop=(kc == KCP - 1),
                        )
                for m in range(2):
                    dst = (qt, kt)[t]
                    nc.scalar.activation(
                        out=dst[:, m, sc * 512 : (sc + 1) * 512],
                        in_=ps[:, m * 512 : (m + 1) * 512],
                        func=AF.Identity,
                        bias=bias_sb[:, t, m : m + 1],
                        scale=1.0,
                    )

        # ---- Phase B: attention, per (sq-chunk, head-pair) ----
        for sc in range(SC):
            mtile = maskpool.tile(
                [128, KCS, 512], dt.bfloat16, tag="mk", name=f"mk{sc}"
            )
            nc.gpsimd.dma_start(
                out=mtile[:, :, :],
                in_=maskT_d[:, sc * 512 : (sc + 1) * 512].rearrange(
                    "(kc p) q -> p kc q", p=128
                ),
            )
            for p in range(2):
                pv = [
                    pvp.tile([HD + 1, 512], dt.float32, tag="pv", name=f"pv{i}")
                    for i in range(2)
                ]
                sums_sb = smalls.tile([1, 2, 512], dt.float32, tag="sums")
                recip_sb = smalls.tile([1, 2, 512], dt.float32, tag="recip")
                for kc2 in range(KCS // 2):
                    pt = ptpool.tile([128, 2, 2, 512], dt.bfloat16, tag="pt")
                    for j in range(2):
                        kc = 2 * kc2 + j
                        ps = psp.tile([128, 1024], dt.float32, tag="ps")
                        nc.tensor.matmul(
                            ps[:, 0:512],
                            lhsT=kt[0:64, p, kc * 128 : (kc + 1) * 128],
                            rhs=qt[0:64, p, sc * 512 : (sc + 1) * 512],
                            start=True,
                            stop=True,
                        )
                        nc.tensor.matmul(
                            ps[:, 512:1024],
                            lhsT=kt[64:128, p, kc * 128 : (kc + 1) * 128],
                            rhs=qt[64:128, p, sc * 512 : (sc + 1) * 512],
                            start=True,
                            stop=True,
                            tile_position=(64, 0),
                        )
                        nc.scalar.activation(
                            out=pt[:, j, :, :].rearrange("s h q -> s (h q)"),
                            in_=ps[:, :],
                            func=AF.Exp,
                            scale=0.125,
                        )
                    for i in range(2):
                        nc.vector.tensor_mul(
                            out=pt[:, :, i, :],
                            in0=pt[:, :, i, :],
                            in1=mtile[:, 2 * kc2 : 2 * kc2 + 2, :],
                        )
                    for j in range(2):
                        kc = 2 * kc2 + j
                        for i in range(2):
                            nc.tensor.matmul(
                                pv[i][:, :],
                                lhsT=vaug[:, kc, 2 * p + i, :],
                                rhs=pt[:, j, i, :],
                                start=(kc == 0),
                                stop=(kc == KCS - 1),
                            )
                # softmax denominators -> reciprocal -> PE broadcast -> normalize
                for i in range(2):
                    nc.vector.tensor_copy(
                        out=sums_sb[0:1, i, :], in_=pv[i][HD : HD + 1, :]
                    )
                if (sc + p) % 2 == 0:
                    nc.scalar.activation(
                        out=recip_sb[0:1, :, :],
                        in_=sums_sb[0:1, :, :],
                        func=AF.Ln,
                    )
                    nc.scalar.activation(
                        out=recip_sb[0:1, :, :],
                        in_=recip_sb[0:1, :, :],
                        func=AF.Exp,
                        scale=-1.0,
                    )
                else:
                    nc.vector.reciprocal(
                        out=recip_sb[0:1, :, :], in_=sums_sb[0:1, :, :]
                    )
                bcs = smalls.tile([128, 1024], dt.float32, tag="bcs")
                nc.gpsimd.partition_broadcast(
                    bcs[:, :], recip_sb[0:1, :, :].rearrange("o h q -> o (h q)")
                )
                for i in range(2):
                    nc.vector.tensor_mul(
                        out=attnT[
                            64 * i : 64 * (i + 1), p, sc * 512 : (sc + 1) * 512
                        ],
                        in0=pv[i][0:HD, :],
                        in1=bcs[0:HD, i * 512 : (i + 1) * 512],
                    )
            # ---- output projection, delayed one chunk so deps are stale ----
            for s1 in ([] if sc == 0 else range(sc * 4 - 4, sc * 4)):
                po = pop.tile([128, 1024], dt.float32, tag="po")
                for c in range(2):
                    for m in range(2):
                        nc.tensor.matmul(
                            po[:, m * 512 : (m + 1) * 512],
                            lhsT=attnT[:, c, s1 * 128 : (s1 + 1) * 128],
                            rhs=wo_sb[:, c, m * 512 : (m + 1) * 512],
                            start=(c == 0),
                            stop=(c == 1),
                        )
                ot = outpool.tile([128, 1024], dt.float32, tag="ot")
                if s1 % 2 == 0:
                    nc.scalar.activation(out=ot[:, :], in_=po[:, :], func=AF.Copy)
                else:
                    nc.vector.tensor_copy(out=ot[:, :], in_=po[:, :])
                nc.gpsimd.dma_start(
                    out=out_d[s1 * 128 : (s1 + 1) * 128, :], in_=ot[:, :]
                )

        for s1 in range((SC - 1) * 4, SC * 4):
            po = pop.tile([128, 1024], dt.float32, tag="po")
            for c in range(2):
                for m in range(2):
                    nc.tensor.matmul(
                        po[:, m * 512 : (m + 1) * 512],
                        lhsT=attnT[:, c, s1 * 128 : (s1 + 1) * 128],
                        rhs=wo_sb[:, c, m * 512 : (m + 1) * 512],
                        start=(c == 0),
                        stop=(c == 1),
                    )
            ot = outpool.tile([128, 1024], dt.float32, tag="ot")
            if s1 % 2 == 0:
                nc.scalar.activation(out=ot[:, :], in_=po[:, :], func=AF.Copy)
            else:
                nc.vector.tensor_copy(out=ot[:, :], in_=po[:, :])
            nc.gpsimd.dma_start(
                out=out_d[s1 * 128 : (s1 + 1) * 128, :], in_=ot[:, :]
            )


    nc.compile()
    return nc


def _prep_inputs(query, key_, value, mask, Wq, bq, Wk, bk, Wv, bv, Wo, bo):
    bf16 = ml_dtypes.bfloat16
    f32 = np.float32
    per_batch = []
    for b in range(B):
        per_batch.append(
            {
                "xqT": np.ascontiguousarray(np.asarray(query[b], f32).T).astype(bf16),
                "xkT": np.ascontiguousarray(np.asarray(key_[b], f32).T).astype(bf16),
                "xvT": np.ascontiguousarray(np.asarray(value[b], f32).T).astype(bf16),
                "maskT": np.ascontiguousarray(np.asarray(mask[b, 0]).T).astype(bf16),
            }
        )
    in_maps = []
    for c in range(NCORES):
        b, hq = divmod(c, NCORES // B)
        cs = slice(DH4 * hq, DH4 * (hq + 1))
        m = dict(per_batch[b])
        m["wq"] = np.ascontiguousarray(np.asarray(Wq, f32)[:, cs]).astype(bf16)
        m["wk"] = np.ascontiguousarray(np.asarray(Wk, f32)[:, cs]).astype(bf16)
        m["wv"] = np.ascontiguousarray(np.asarray(Wv, f32)[:, cs]).astype(bf16)
        m["bqkv"] = np.ascontiguousarray(
            np.stack([np.asarray(bq, f32)[cs], np.asarray(bk, f32)[cs]])
        )
        m["wo"] = np.ascontiguousarray(np.asarray(Wo, f32)[cs, :]).astype(bf16)
        in_maps.append(m)
    return in_maps


def kernel(query, key_, value, mask, Wq, bq, Wk, bk, Wv, bv, Wo, bo):
    from concourse.bass_utils import run_bass_kernel_spmd

    if "nc" not in _CACHE:
        _CACHE["nc"] = _build_nc()
    nc = _CACHE["nc"]

    in_maps = _prep_inputs(
        query, key_, value, mask, Wq, bq, Wk, bk, Wv, bv, Wo, bo
    )
    res = run_bass_kernel_spmd(nc, in_maps, core_ids=list(range(NCORES))).results

    out = np.zeros((B, S, D), np.float32)
    for c in range(NCORES):
        out[c // (NCORES // B)] += res[c]["out"]
    out += (
        np.asarray(bv, np.float32) @ np.asarray(Wo, np.float32)
        + np.asarray(bo, np.float32)
    )[None, None, :]
    return out


# revision 16
# speedup vs baseline: 1.6209x; 1.0941x over previous
"""Multi-head attention (B=2, S=2048, D=1024, H=16) on 8 Trainium2 NeuronCores.

Sharding: batch x head-group. Core c handles batch c//4 and heads 4*(c%4)..4*(c%4)+3
(column-parallel Wq/Wk/Wv, row-parallel Wo; partial outputs summed on host).

Per-core dataflow (all in "transposed" orientation so the PE contracts naturally):
  Q^T/K^T = W^T @ X^T   (f32r matmuls, full fp32 data at 1 cyc/row)  -> bf16 SBUF
  V^T     = Wv^T @ Xv^T -> PE-transpose -> V natural [s, hd] (+ ones col for sums)
  scores^T[sk,sq] = K_h @ Q_h^T  (bf16, two heads row-packed in the 128-wide PE)
  P^T = exp(scores^T/8) (ACT, psum->sbuf bf16), masked by maskT (DVE bf16 2x)
  attn^T[hd+1,sq] = [V_h|1]^T @ P^T  (ones row accumulates softmax denominators)
  normalize via PE ones-broadcast of 1/sums + DVE multiply -> attnT bf16
  out[sq,do] = attnT^T @ Wo  (bf16) -> DMA out.
"""

import numpy as np
import ml_dtypes

B, S, D, H, HD = 2, 2048, 1024, 16, 64
NCORES = 8
HPC = 4          # heads per core
DH4 = HPC * HD   # 256 projection cols per core
KCP = D // 128   # 8 contraction chunks for projections
SC = S // 512    # 4 sq chunks
KCS = S // 128   # 16 sk chunks

_CACHE = {}


def _build_nc():
    from contextlib import ExitStack

    import concourse.bacc as bacc
    import concourse.tile as tile
    from concourse import mybir
    
    dt = mybir.dt
    AF = mybir.ActivationFunctionType

    nc = bacc.Bacc("TRN2", target_bir_lowering=False, debug=False)

    xT = [
        nc.dram_tensor(n, [D, S], dt.bfloat16, kind="ExternalInput")
        for n in ("xqT", "xkT", "xvT")
    ]
    maskT_d = nc.dram_tensor("maskT", [S, S], dt.bfloat16, kind="ExternalInput")
    w_d = [
        nc.dram_tensor(n, [D, DH4], dt.bfloat16, kind="ExternalInput")
        for n in ("wq", "wk", "wv")
    ]
    bqkv_d = nc.dram_tensor("bqkv", [2, DH4], dt.float32, kind="ExternalInput")
    wo_d = nc.dram_tensor("wo", [DH4, D], dt.bfloat16, kind="ExternalInput")
    out_d = nc.dram_tensor("out", [S, D], dt.float32, kind="ExternalOutput")

    with tile.TileContext(nc) as tc, ExitStack() as ctx:
        consts = ctx.enter_context(tc.tile_pool(name="consts", bufs=1))
        wpool = ctx.enter_context(tc.tile_pool(name="wpool", bufs=1))
        persist = ctx.enter_context(tc.tile_pool(name="persist", bufs=1))
        xtpool = ctx.enter_context(tc.tile_pool(name="xtpool", bufs=2))
        xvpool = ctx.enter_context(tc.tile_pool(name="xvpool", bufs=2))
        maskpool = ctx.enter_context(tc.tile_pool(name="maskpool", bufs=2))
        ptpool = ctx.enter_context(tc.tile_pool(name="ptpool", bufs=8))
        smalls = ctx.enter_context(tc.tile_pool(name="smalls", bufs=2))
        outpool = ctx.enter_context(tc.tile_pool(name="outpool", bufs=2))
        psp = ctx.enter_context(tc.tile_pool(name="psp", bufs=2, space="PSUM"))
        pvp = ctx.enter_context(tc.tile_pool(name="pvp", bufs=2, space="PSUM"))

        bias_sb = consts.tile([128, 2, 2], dt.float32)
        nc.sync.dma_start(
            out=bias_sb[:, :, :],
            in_=bqkv_d[:, :].rearrange("t (m p) -> p t m", p=128),
        )
        wo_sb = consts.tile([128, 2, D], dt.bfloat16)
        nc.sync.dma_start(
            out=wo_sb[:, :, :], in_=wo_d[:, :].rearrange("(c p) d -> p c d", p=128)
        )

        w_sb = wpool.tile([128, KCP, 3, DH4], dt.bfloat16, tag="w")
        for t in range(3):
            nc.sync.dma_start(
                out=w_sb[:, :, t, :],
                in_=w_d[t][:, :].rearrange("(kc p) m -> p kc m", p=128),
            )

        qt = persist.tile([128, 2, S], dt.bfloat16, tag="qt")
        kt = persist.tile([128, 2, S], dt.bfloat16, tag="kt")
        vaug = persist.tile([128, KCS, HPC, HD + 1], dt.bfloat16, tag="vaug")
        attnT = persist.tile([128, 2, S], dt.bfloat16, tag="attnT")
        nc.vector.memset(vaug[:, :, :, HD : HD + 1], 1.0)

        # ---- Phase A: K^T, Q^T projections (rhs = X^T tiles) and
        # V in natural layout directly (lhsT = Xv^T tiles, stationary) ----
        for t in (1, 2, 0):
            dma_eng = {1: nc.sync, 2: nc.scalar, 0: nc.sync}[t]
            if t == 2:
                for sc in range(SC):
                    xv_t = xvpool.tile(
                        [128, KCP, 512], dt.bfloat16, tag="xv", name=f"xv{sc}"
                    )
                    dma_eng.dma_start(
                        out=xv_t[:, :, :],
                        in_=xT[t][:, sc * 512 : (sc + 1) * 512].rearrange(
                            "(kc p) q -> p kc q", p=128
                        ),
                    )
                    for j in range(4):
                        po_v = pvp.tile([128, DH4], dt.float32, tag="pv")
                        for kc in range(KCP):
                            nc.tensor.matmul(
                                po_v[:, :],
                                lhsT=xv_t[:, kc, j * 128 : (j + 1) * 128],
                                rhs=w_sb[:, kc, 2, :],
                                start=(kc == 0),
                                stop=(kc == KCP - 1),
                            )
                        nc.vector.tensor_copy(
                            out=vaug[:, sc * 4 + j, :, 0:HD],
                            in_=po_v[:, :].rearrange("p (h d) -> p h d", h=4),
                        )
                continue
            for sc in range(SC):
                ps = psp.tile([128, 1024], dt.float32, tag="ps")
                xt_t = xtpool.tile([128, KCP, 512], dt.bfloat16, tag="xt")
                dma_eng.dma_start(
                    out=xt_t[:, :, :],
                    in_=xT[t][:, sc * 512 : (sc + 1) * 512].rearrange(
                        "(kc p) q -> p kc q", p=128
                    ),
                )
                for kc in range(KCP):
                    for m in range(2):
                        nc.tensor.matmul(
                            ps[:, m * 512 : (m + 1) * 512],
                            lhsT=w_sb[:, kc, t, m * 128 : (m + 1) * 128],
                            rhs=xt_t[:, kc, :],
                            start=(kc == 0),
                            stop=(kc == KCP - 1),
                        )
                for m in range(2):
                    dst = (qt, kt)[t]
                    nc.scalar.activation(
                        out=dst[:, m, sc * 512 : (sc + 1) * 512],
                        in_=ps[:, m * 512 : (m + 1) * 512],
                        func=AF.Identity,
                        bias=bias_sb[:, t, m : m + 1],
                        scale=1.0,
                    )

        # ---- Phase B: attention, per (sq-chunk, head-pair) ----
        for sc in range(SC):
            mtile = maskpool.tile(
                [128, KCS, 512], dt.bfloat16, tag="mk", name=f"mk{sc}"
            )
            nc.gpsimd.dma_start(
                out=mtile[:, :, :],
                in_=maskT_d[:, sc * 512 : (sc + 1) * 512].rearrange(
                    "(kc p) q -> p kc q", p=128
                ),
            )
            for p in range(2):
                pv = pvp.tile([HD + 1, 1024], dt.float32, tag="pv")
                sums_sb = smalls.tile([1, 1024], dt.float32, tag="sums")
                recip_sb = smalls.tile([1, 1024], dt.float32, tag="recip")
                for kc2 in range(KCS // 2):
                    pt = ptpool.tile([128, 2, 2, 512], dt.bfloat16, tag="pt")
                    for j in range(2):
                        kc = 2 * kc2 + j
                        ps = psp.tile([128, 1024], dt.float32, tag="ps")
                        nc.tensor.matmul(
                            ps[:, 0:512],
                            lhsT=kt[0:64, p, kc * 128 : (kc + 1) * 128],
                            rhs=qt[0:64, p, sc * 512 : (sc + 1) * 512],
                            start=True,
                            stop=True,
                        )
                        nc.tensor.matmul(
                            ps[:, 512:1024],
                            lhsT=kt[64:128, p, kc * 128 : (kc + 1) * 128],
                            rhs=qt[64:128, p, sc * 512 : (sc + 1) * 512],
                            start=True,
                            stop=True,
                            tile_position=(64, 0),
                        )
                        nc.scalar.activation(
                            out=pt[:, j, :, :].rearrange("s h q -> s (h q)"),
                            in_=ps[:, :],
                            func=AF.Exp,
                            scale=0.125,
                        )
                    for i in range(2):
                        nc.vector.tensor_mul(
                            out=pt[:, :, i, :],
                            in0=pt[:, :, i, :],
                            in1=mtile[:, 2 * kc2 : 2 * kc2 + 2, :],
                        )
                    for j in range(2):
                        kc = 2 * kc2 + j
                        for i in range(2):
                            nc.tensor.matmul(
                                pv[:, i * 512 : (i + 1) * 512],
                                lhsT=vaug[:, kc, 2 * p + i, :],
                                rhs=pt[:, j, i, :],
                                start=(kc == 0),
                                stop=(kc == KCS - 1),
                            )
                # softmax denominators -> 1/s = exp(-ln(s)) -> broadcast -> normalize
                if (sc + p) % 2 == 0:
                    nc.vector.tensor_copy(
                        out=sums_sb[0:1, :], in_=pv[HD : HD + 1, :]
                    )
                else:
                    nc.scalar.activation(
                        out=sums_sb[0:1, :], in_=pv[HD : HD + 1, :], func=AF.Copy
                    )
                nc.scalar.activation(
                    out=recip_sb[0:1, :], in_=sums_sb[0:1, :], func=AF.Ln
                )
                nc.scalar.activation(
                    out=recip_sb[0:1, :],
                    in_=recip_sb[0:1, :],
                    func=AF.Exp,
                    scale=-1.0,
                )
                bcs = smalls.tile([128, 1024], dt.float32, tag="bcs")
                nc.gpsimd.partition_broadcast(bcs[:, :], recip_sb[0:1, :])
                for i in range(2):
                    nc.vector.tensor_mul(
                        out=attnT[
                            64 * i : 64 * (i + 1), p, sc * 512 : (sc + 1) * 512
                        ],
                        in0=pv[0:HD, i * 512 : (i + 1) * 512],
                        in1=bcs[0:HD, i * 512 : (i + 1) * 512],
                    )
            # ---- output projection, delayed one chunk so deps are stale ----
            for s1 in ([] if sc == 0 else range(sc * 4 - 4, sc * 4)):
                po = psp.tile([128, 1024], dt.float32, tag="ps")
                for c in range(2):
                    for m in range(2):
                        nc.tensor.matmul(
                            po[:, m * 512 : (m + 1) * 512],
                            lhsT=attnT[:, c, s1 * 128 : (s1 + 1) * 128],
                            rhs=wo_sb[:, c, m * 512 : (m + 1) * 512],
                            start=(c == 0),
                            stop=(c == 1),
                        )
                ot = outpool.tile([128, 1024], dt.float32, tag="ot")
                if s1 % 2 == 0:
                    nc.scalar.activation(out=ot[:, :], in_=po[:, :], func=AF.Copy)
                else:
                    nc.vector.tensor_copy(out=ot[:, :], in_=po[:, :])
                nc.gpsimd.dma_start(
                    out=out_d[s1 * 128 : (s1 + 1) * 128, :], in_=ot[:, :]
                )

        for s1 in range((SC - 1) * 4, SC * 4):
            po = psp.tile([128, 1024], dt.float32, tag="ps")
            for c in range(2):
                for m in range(2):
                    nc.tensor.matmul(
                        po[:, m * 512 : (m + 1) * 512],
                        lhsT=attnT[:, c, s1 * 128 : (s1 + 1) * 128],
                        rhs=wo_sb[:, c, m * 512 : (m + 1) * 512],
                        start=(c == 0),
                        stop=(c == 1),
                    )
            ot = outpool.tile([128, 1024], dt.float32, tag="ot")
            if s1 % 2 == 0:
                nc.scalar.activation(out=ot[:, :], in_=po[:, :], func=AF.Copy)
            else:
                nc.vector.tensor_copy(out=ot[:, :], in_=po[:, :])
            nc.gpsimd.dma_start(
                out=out_d[s1 * 128 : (s1 + 1) * 128, :], in_=ot[:, :]
            )


    nc.compile()
    return nc


def _prep_inputs(query, key_, value, mask, Wq, bq, Wk, bk, Wv, bv, Wo, bo):
    bf16 = ml_dtypes.bfloat16
    f32 = np.float32
    per_batch = []
    for b in range(B):
        per_batch.append(
            {
                "xqT": np.ascontiguousarray(np.asarray(query[b], f32).T).astype(bf16),
                "xkT": np.ascontiguousarray(np.asarray(key_[b], f32).T).astype(bf16),
                "xvT": np.ascontiguousarray(np.asarray(value[b], f32).T).astype(bf16),
                "maskT": np.ascontiguousarray(np.asarray(mask[b, 0]).T).astype(bf16),
            }
        )
    in_maps = []
    for c in range(NCORES):
        b, hq = divmod(c, NCORES // B)
        cs = slice(DH4 * hq, DH4 * (hq + 1))
        m = dict(per_batch[b])
        m["wq"] = np.ascontiguousarray(np.asarray(Wq, f32)[:, cs]).astype(bf16)
        m["wk"] = np.ascontiguousarray(np.asarray(Wk, f32)[:, cs]).astype(bf16)
        m["wv"] = np.ascontiguousarray(np.asarray(Wv, f32)[:, cs]).astype(bf16)
        m["bqkv"] = np.ascontiguousarray(
            np.stack([np.asarray(bq, f32)[cs], np.asarray(bk, f32)[cs]])
        )
        m["wo"] = np.ascontiguousarray(np.asarray(Wo, f32)[cs, :]).astype(bf16)
        in_maps.append(m)
    return in_maps


def kernel(query, key_, value, mask, Wq, bq, Wk, bk, Wv, bv, Wo, bo):
    from concourse.bass_utils import run_bass_kernel_spmd

    if "nc" not in _CACHE:
        _CACHE["nc"] = _build_nc()
    nc = _CACHE["nc"]

    in_maps = _prep_inputs(
        query, key_, value, mask, Wq, bq, Wk, bk, Wv, bv, Wo, bo
    )
    res = run_bass_kernel_spmd(nc, in_maps, core_ids=list(range(NCORES))).results

    out = np.zeros((B, S, D), np.float32)
    for c in range(NCORES):
        out[c // (NCORES // B)] += res[c]["out"]
    out += (
        np.asarray(bv, np.float32) @ np.asarray(Wo, np.float32)
        + np.asarray(bo, np.float32)
    )[None, None, :]
    return out
